# revision 6
# baseline (speedup 1.0000x reference)
# Trainium2 Bass kernel for nn_MultiHeadAttention_77120432767767
# (relative-position sparse attention).
#
# Sharding: batch (2) x head-pairs (4) across 8 cores. Core c handles
# batch b=c//4, heads h0=2*(c%4), h0+1. base_rpr / projection weights are
# folded on the host into per-core weight slices; dist is sharded with batch.
#
# Key trick: the relative-position gather attn2[i,j] = qb[i, min(dist[i,j],5)]
# is evaluated as a degree-5 polynomial in w = (min(dist,5) - 2.5)/2.5
# (Chebyshev-style nodes in [-1,1] so fp16 rounding does not amplify), whose
# per-row coefficients come out of the q-projection matmul for free: the
# Vandermonde inverse is folded into the projection weights on the host.
import sys
import os

sys.path.insert(0, '/opt/trn_rl_repo')

import numpy as np
from contextlib import ExitStack

import concourse.bass as bass
import concourse.tile as tile
from concourse import mybir
from concourse.bass_utils import run_bass_kernel_spmd

F32 = mybir.dt.float32
F16 = mybir.dt.float16
I32 = mybir.dt.int32
AL = mybir.AluOpType
AF = mybir.ActivationFunctionType

SEQ, DM, NH, DK, M1 = 1024, 512, 8, 64, 6
HPC = 2           # heads per core
NT = SEQ // 128   # 8 seq tiles
NDT = DM // 128   # 4 model-dim tiles

# ---------------------------------------------------------------------------
# walrus in this container only accepts ONE sync-wait per ctrl instruction;
# TileContext's tail drain attaches one wait per outstanding semaphore.
# Spread them across single-wait NOPs on SP instead.
_orig_drain_and_barrier = tile.TileContext._drain_and_barrier


def _patched_drain_and_barrier(self, tick_clock, wait_clock):
    collector = self.nc.sync.nop()
    wait_clock.add_sem_waits(
        collector.ins, tile.ScopedClock({None: tick_clock.global_clock})
    )
    si = collector.ins.sync_info
    waits = list(si.on_wait or []) if si else []
    if si:
        si.on_wait = waits[:1]
    for i in range(1, len(waits)):
        n = self.nc.sync.nop()
        n.ins.sync_info = mybir.SyncInfo(on_wait=[waits[i]], on_update=[])
    self.nc.sync.drain()
    self.nc.all_engine_barrier()
    popped = self.nc._tile_sem_poison_stack.pop()
    assert popped is self._sem_poison
    self.nc.clear_and_free_semaphores(list(self.sems.allocated().values()))
    self.nc.all_engine_barrier()


tile.TileContext._drain_and_barrier = _patched_drain_and_barrier

_wsplit_ctr = [0]


def _split_multiwait_instructions(nc):
    """walrus here accepts one sync-wait per instruction; hoist extras onto
    same-engine NOPs inserted immediately before (same stream => same order)."""
    for bassbb in nc.bb_map.values():
        bb = bassbb.bb
        insts = list(bb.instructions)
        new, changed = [], False
        for inst in insts:
            si = inst.sync_info
            if si is not None and si.on_wait and len(si.on_wait) > 1:
                waits = list(si.on_wait)
                for w in waits[:-1]:
                    nop = mybir.InstNoOp(
                        name=f"I-wsplit{_wsplit_ctr[0]}", ins=[], outs=[])
                    _wsplit_ctr[0] += 1
                    nop.engine = inst.engine
                    nop.sync_info = mybir.SyncInfo(on_wait=[w], on_update=[])
                    nc.register_instruction(nop)
                    new.append(nop)
                si.on_wait = [waits[-1]]
                changed = True
            new.append(inst)
        if changed:
            bb.instructions = new


# ---------------------------------------------------------------------------
def build_program(use_mask: bool, use_beta: bool):
    nc = bass.Bass()
    qx = nc.declare_dram_parameter("qx", [SEQ, DM], F32, isOutput=False)
    kx = nc.declare_dram_parameter("kx", [SEQ, DM], F32, isOutput=False)
    vx = nc.declare_dram_parameter("vx", [SEQ, DM], F32, isOutput=False)
    dist = nc.declare_dram_parameter("dist", [SEQ, SEQ], I32, isOutput=False)
    wq = nc.declare_dram_parameter("wq", [DM, 128 + HPC * M1], F16, isOutput=False)
    wk = nc.declare_dram_parameter("wk", [DM, 128], F16, isOutput=False)
    wv = nc.declare_dram_parameter("wv", [DM, 128], F16, isOutput=False)
    wfc = nc.declare_dram_parameter("wfc", [128, DM], F16, isOutput=False)
    id16 = nc.declare_dram_parameter("id16", [128, 128], F16, isOutput=False)
    id32 = nc.declare_dram_parameter("id32", [128, 128], F32, isOutput=False)
    maskp = None
    if use_mask:
        maskp = nc.declare_dram_parameter("maskp", [SEQ, SEQ], I32, isOutput=False)
    qbias = None
    if use_beta:
        # beta @ wq_ext : per-output-channel bias for qs rows / coef rows
        qbias = nc.declare_dram_parameter(
            "qbias", [128 + HPC * M1, 1], F32, isOutput=False
        )

    attn_o = nc.declare_dram_parameter("attn_o", [HPC, SEQ, SEQ], F32, isOutput=True)
    out_o = nc.declare_dram_parameter("out_o", [DM, SEQ], F32, isOutput=True)

    with tile.TileContext(nc) as tc, ExitStack() as ctx:
        body(ctx, tc, qx, kx, vx, dist, wq, wk, wv, wfc, id16, id32,
             maskp, qbias, attn_o, out_o)
    _split_multiwait_instructions(nc)
    return nc


def body(ctx, tc, qx, kx, vx, dist, wq, wk, wv, wfc, id16, id32,
         maskp, qbias, attn_o, out_o):
    nc = tc.nc

    singles = ctx.enter_context(tc.tile_pool(name="singles", bufs=1))
    ld = ctx.enter_context(tc.tile_pool(name="ld", bufs=3))          # f32 input tiles
    cv = ctx.enter_context(tc.tile_pool(name="cv", bufs=3))          # f16 converted tiles
    stats = ctx.enter_context(tc.tile_pool(name="stats", bufs=4))
    big = ctx.enter_context(tc.tile_pool(name="big", bufs=1))        # resident tensors
    work = ctx.enter_context(tc.tile_pool(name="work", bufs=3))      # horner/x/w tiles
    pwork = ctx.enter_context(tc.tile_pool(name="pwork", bufs=3))    # pu / pn tiles
    # PSUM budget: psA "psbig" [128,1024]f32 = 2 banks x 2 bufs = 4 banks;
    # psB "pssmall" up to 4KB/partition = 2 banks x 2 bufs = 4 banks. Total 8.
    psA = ctx.enter_context(tc.tile_pool(name="psA", bufs=2, space="PSUM"))
    psB = ctx.enter_context(tc.tile_pool(name="psB", bufs=2, space="PSUM"))

    # identities
    ident16 = singles.tile([128, 128], F16, tag="ident16")
    nc.sync.dma_start(out=ident16, in_=id16[:, :])
    ident32 = singles.tile([128, 128], F32, tag="ident32")
    nc.sync.dma_start(out=ident32, in_=id32[:, :])

    # weights: [128, 4, E] layout: partition = dm % 128, free = (dm//128, e)
    wq_sb = singles.tile([128, NDT, 128 + HPC * M1], F16, tag="wq")
    nc.sync.dma_start(out=wq_sb, in_=wq.rearrange("(t p) e -> p t e", p=128))
    wk_sb = singles.tile([128, NDT, 128], F16, tag="wk")
    nc.sync.dma_start(out=wk_sb, in_=wk.rearrange("(t p) e -> p t e", p=128))
    wv_sb = singles.tile([128, NDT, 128], F16, tag="wv")
    nc.sync.dma_start(out=wv_sb, in_=wv.rearrange("(t p) e -> p t e", p=128))
    wfc_sb = singles.tile([128, DM], F16, tag="wfc")
    nc.sync.dma_start(out=wfc_sb, in_=wfc[:, :])
    qbias_sb = None
    if qbias is not None:
        qbias_sb = singles.tile([128 + HPC * M1, 1], F32, tag="qbias")
        nc.sync.dma_start(out=qbias_sb, in_=qbias[:, :])

    # resident transposed activations: [128, dm_tile, seq]
    qnT = big.tile([128, NDT, SEQ], F16, tag="qnT")
    kT = big.tile([128, NDT, SEQ], F16, tag="kT")
    vT = big.tile([128, NDT, SEQ], F16, tag="vT")

    # ---- load q, LN stats, qn16, transpose ----
    for t in range(NT):
        qt = ld.tile([128, DM], F32, tag="qld")
        nc.sync.dma_start(out=qt, in_=qx[t * 128:(t + 1) * 128, :])
        st = stats.tile([128, nc.vector.BN_STATS_DIM], F32, tag="bst")
        nc.vector.bn_stats(out=st, in_=qt)
        mv = stats.tile([128, nc.vector.BN_AGGR_DIM], F32, tag="bagg")
        nc.vector.bn_aggr(out=mv, in_=st)
        varp = stats.tile([128, 1], F32, tag="varp")
        nc.vector.tensor_scalar_add(varp, mv[:, 1:2], 1e-6)
        sd = stats.tile([128, 1], F32, tag="sd")
        nc.scalar.sqrt(sd, varp)
        rstd = stats.tile([128, 1], F32, tag="rstd")
        nc.vector.reciprocal(rstd, sd)
        qn = cv.tile([128, DM], F16, tag="qn")
        nc.vector.tensor_scalar(qn, qt, mv[:, 0:1], rstd, AL.subtract, AL.mult)
        # transpose 4 chunks -> qnT[:, :, t*128...]
        pt = psB.tile([128, DM], F16, tag="pssmall")
        for c in range(NDT):
            nc.tensor.transpose(pt[:, c * 128:(c + 1) * 128],
                                qn[:, c * 128:(c + 1) * 128], ident16)
        nc.scalar.copy(qnT[:, :, t * 128:(t + 1) * 128],
                       pt.rearrange("p (c s) -> p c s", c=NDT))

    # ---- load k, v, convert, transpose ----
    for t in range(NT):
        kt = ld.tile([128, DM], F32, tag="kld")
        nc.sync.dma_start(out=kt, in_=kx[t * 128:(t + 1) * 128, :])
        k16 = cv.tile([128, DM], F16, tag="k16")
        nc.scalar.copy(k16, kt)
        pt = psB.tile([128, DM], F16, tag="pssmall")
        for c in range(NDT):
            nc.tensor.transpose(pt[:, c * 128:(c + 1) * 128],
                                k16[:, c * 128:(c + 1) * 128], ident16)
        nc.scalar.copy(kT[:, :, t * 128:(t + 1) * 128],
                       pt.rearrange("p (c s) -> p c s", c=NDT))
    for t in range(NT):
        vt = ld.tile([128, DM], F32, tag="vld")
        nc.sync.dma_start(out=vt, in_=vx[t * 128:(t + 1) * 128, :])
        v16 = cv.tile([128, DM], F16, tag="v16")
        nc.scalar.copy(v16, vt)
        pt = psB.tile([128, DM], F16, tag="pssmall")
        for c in range(NDT):
            nc.tensor.transpose(pt[:, c * 128:(c + 1) * 128],
                                v16[:, c * 128:(c + 1) * 128], ident16)
        nc.scalar.copy(vT[:, :, t * 128:(t + 1) * 128],
                       pt.rearrange("p (c s) -> p c s", c=NDT))

    # ---- projections ----
    NE = 128 + HPC * M1
    # q projection (+ polynomial coefficient columns)
    ps_q = psA.tile([128, SEQ], F32, tag="psbig")
    ps_c = psA.tile([M1 * HPC, SEQ], F32, tag="psbig")
    for c in range(NDT):
        for half in range(2):
            sl = slice(half * 512, (half + 1) * 512)
            nc.tensor.matmul(ps_q[:, sl], wq_sb[:, c, 0:128], qnT[:, c, sl],
                             start=(c == 0), stop=(c == NDT - 1))
            nc.tensor.matmul(ps_c[:, sl], wq_sb[:, c, 128:NE], qnT[:, c, sl],
                             start=(c == 0), stop=(c == NDT - 1))
    qsT = big.tile([128, SEQ], F16, tag="qsT")
    if qbias_sb is not None:
        nc.vector.tensor_scalar_add(qsT, ps_q, qbias_sb[0:128, :])
    else:
        nc.vector.tensor_copy(qsT, ps_q)
    aTf = big.tile([M1 * HPC, SEQ], F32, tag="aTf")
    if qbias_sb is not None:
        nc.vector.tensor_scalar_add(aTf, ps_c, qbias_sb[128:NE, :])
    else:
        nc.vector.tensor_copy(aTf, ps_c)
    # coefficients natural layout [seq, 12]
    a_sb = big.tile([128, NT, M1 * HPC], F32, tag="a_sb")
    for t in range(NT):
        pc = psB.tile([128, M1 * HPC], F32, tag="pssmall")
        nc.tensor.transpose(pc, aTf[:, t * 128:(t + 1) * 128],
                            ident32[0:M1 * HPC, 0:M1 * HPC])
        nc.vector.tensor_copy(a_sb[:, t, :], pc)

    # k projection
    ps_k = psA.tile([128, SEQ], F32, tag="psbig")
    for c in range(NDT):
        for half in range(2):
            sl = slice(half * 512, (half + 1) * 512)
            nc.tensor.matmul(ps_k[:, sl], wk_sb[:, c, :], kT[:, c, sl],
                             start=(c == 0), stop=(c == NDT - 1))
    khT = big.tile([128, SEQ], F16, tag="khT")
    nc.vector.tensor_copy(khT, ps_k)

    # v projection -> vh natural [seq, 128] stored as [128, jt, 128]
    vh = big.tile([128, NT, 128], F16, tag="vh")
    for jt in range(NT):
        ps_v = psB.tile([128, 128], F32, tag="pssmall")
        for c in range(NDT):
            nc.tensor.matmul(ps_v, vT[:, c, jt * 128:(jt + 1) * 128],
                             wv_sb[:, c, :],
                             start=(c == 0), stop=(c == NDT - 1))
        nc.vector.tensor_copy(vh[:, jt, :], ps_v)

    # ---- main attention loop ----
    c04 = float(np.float16(0.4))
    pT = [big.tile([128, NT, SEQ], F16, tag=f"pT{h}", name=f"pT{h}") for h in range(HPC)]
    rstore = [big.tile([128, NT], F32, tag=f"rst{h}", name=f"rst{h}") for h in range(HPC)]

    for i in range(NT):
        isl = slice(i * 128, (i + 1) * 128)
        dt_ = ld.tile([128, SEQ], I32, tag="dld")
        nc.sync.dma_start(out=dt_, in_=dist[isl, :])
        x16 = work.tile([128, SEQ], F16, tag="x16")
        nc.gpsimd.tensor_scalar_min(x16, dt_, 5)
        w16 = work.tile([128, SEQ], F16, tag="w16")
        nc.gpsimd.tensor_scalar(w16, x16, c04, 1.0, AL.mult, AL.subtract)
        mk = None
        if maskp is not None:
            mt = ld.tile([128, SEQ], I32, tag="mld")
            nc.sync.dma_start(out=mt, in_=maskp[isl, :])
            mk = work.tile([128, SEQ], F16, tag="mk16")
            # (mask * 1e9) - 1e9 -> 0 where mask==1, -1e9 where mask==0
            nc.gpsimd.tensor_scalar(mk, mt, 1e9, 1e9, AL.mult, AL.subtract)

        for h in range(HPC):
            hsl = slice(h * 64, (h + 1) * 64)
            ac = lambda j: a_sb[:, i, h * M1 + j:h * M1 + j + 1]
            ps_l = psA.tile([128, SEQ], F32, tag="psbig")
            for half in range(2):
                sl = slice(half * 512, (half + 1) * 512)
                nc.tensor.matmul(ps_l[:, sl], qsT[hsl, isl], khT[hsl, sl],
                                 start=True, stop=False, skip_group_check=True)
            # Horner: u = a5*w + a4 ; u = (u+s)*w for s in (0, a3, a2, a1)
            u = work.tile([128, SEQ], F16, tag="u")
            nc.vector.tensor_scalar(u, w16, ac(5), ac(4), AL.mult, AL.add)
            for s in (0.0, 3, 2, 1):
                un = work.tile([128, SEQ], F16, tag="u")
                sc = s if isinstance(s, float) else ac(s)
                nc.vector.scalar_tensor_tensor(un, u, sc, w16, AL.add, AL.mult)
                u = un
            for half in range(2):
                sl = slice(half * 512, (half + 1) * 512)
                nc.tensor.matmul(ps_l[:, sl], ident16, u[:, sl],
                                 start=False, stop=(mk is None),
                                 skip_group_check=True)
                if mk is not None:
                    nc.tensor.matmul(ps_l[:, sl], ident16, mk[:, sl],
                                     start=False, stop=True,
                                     skip_group_check=True)
            pu = pwork.tile([128, SEQ], F16, tag="pu")
            sums = stats.tile([128, 1], F32, tag="sums")
            nc.scalar.activation(pu, ps_l, AF.Exp, bias=ac(0), scale=1.0,
                                 accum_out=sums)
            nc.vector.reciprocal(rstore[h][:, i:i + 1], sums)
            pn = pwork.tile([128, SEQ], F32, tag="pn")
            nc.scalar.activation(pn, pu, AF.Copy, bias=0.0,
                                 scale=rstore[h][:, i:i + 1])
            nc.gpsimd.dma_start(out=attn_o[h, isl, :], in_=pn)
            # transpose pu -> pT[h][:, jt, i*128...]
            for g in range(2):
                pt = psB.tile([128, 512], F16, tag="pssmall")
                for c in range(4):
                    j0 = (g * 4 + c) * 128
                    nc.tensor.transpose(pt[:, c * 128:(c + 1) * 128],
                                        pu[:, j0:j0 + 128], ident16)
                if g == 0:
                    nc.vector.tensor_copy(
                        pT[h][:, g * 4:(g + 1) * 4, isl],
                        pt.rearrange("p (c s) -> p c s", c=4))
                else:
                    nc.scalar.copy(
                        pT[h][:, g * 4:(g + 1) * 4, isl],
                        pt.rearrange("p (c s) -> p c s", c=4))

    # ---- PV + normalize + FC ----
    ones64 = singles.tile([1, 64], F32, tag="ones64")
    nc.vector.memset(ones64, 1.0)
    aoT = big.tile([128, SEQ], F16, tag="aoT")
    for h in range(HPC):
        hsl = slice(h * 64, (h + 1) * 64)
        # rT: transpose rstore[h] columns -> [1, SEQ]
        ps_rt = psB.tile([1, SEQ], F32, tag="pssmall")
        for t in range(NT):
            nc.tensor.transpose(ps_rt[0:1, t * 128:(t + 1) * 128],
                                rstore[h][:, t:t + 1], ident32)
        rT = stats.tile([1, SEQ], F32, tag="rT")
        nc.vector.tensor_copy(rT, ps_rt)
        ps_R = psA.tile([64, SEQ], F32, tag="psbig")
        for half in range(2):
            sl = slice(half * 512, (half + 1) * 512)
            nc.tensor.matmul(ps_R[:, sl], ones64, rT[:, sl],
                             start=True, stop=True)
        Rh = pwork.tile([64, SEQ], F32, tag="Rh")
        nc.vector.tensor_copy(Rh, ps_R)
        ps_ao = psA.tile([64, SEQ], F32, tag="psbig")
        for jt in range(NT):
            for half in range(2):
                sl = slice(half * 512, (half + 1) * 512)
                nc.tensor.matmul(ps_ao[:, sl], vh[:, jt, hsl],
                                 pT[h][:, jt, sl],
                                 start=(jt == 0), stop=(jt == NT - 1))
        nc.vector.tensor_mul(aoT[hsl, :], ps_ao, Rh)

    for sl4 in range(NDT):
        ps_f = psA.tile([128, SEQ], F32, tag="psbig")
        for half in range(2):
            sl = slice(half * 512, (half + 1) * 512)
            nc.tensor.matmul(ps_f[:, sl], wfc_sb[:, sl4 * 128:(sl4 + 1) * 128],
                             aoT[:, sl], start=True, stop=True)
        of = pwork.tile([128, SEQ], F32, tag="of")
        nc.scalar.copy(of, ps_f)
        nc.gpsimd.dma_start(out=out_o[sl4 * 128:(sl4 + 1) * 128, :], in_=of)


# ---------------------------------------------------------------------------
_PROGRAM_CACHE = {}


def _get_program(use_mask, use_beta):
    key = (use_mask, use_beta)
    if key not in _PROGRAM_CACHE:
        _PROGRAM_CACHE[key] = build_program(use_mask, use_beta)
    return _PROGRAM_CACHE[key]


def kernel(q, k, v, mask, dist, w_qs, w_ks, w_vs, w_fc, ln_gamma, ln_beta,
           base_rpr):
    q = np.asarray(q, np.float32)
    k = np.asarray(k, np.float32)
    v = np.asarray(v, np.float32)
    mask = np.asarray(mask, np.int32)
    dist = np.asarray(dist, np.int32)
    w_qs = np.asarray(w_qs, np.float32)
    w_ks = np.asarray(w_ks, np.float32)
    w_vs = np.asarray(w_vs, np.float32)
    w_fc = np.asarray(w_fc, np.float32)
    ln_gamma = np.asarray(ln_gamma, np.float32)
    ln_beta = np.asarray(ln_beta, np.float32)
    base_rpr = np.asarray(base_rpr, np.float32)

    BS = q.shape[0]
    use_mask = not bool(np.all(mask == 1))
    use_beta = bool(np.any(ln_beta != 0.0))

    # device-exact Chebyshev-ish nodes: w_m = fp16(fp16(m) * fp16(0.4) - 1)
    c04 = np.float32(np.float16(0.4))
    nodes = np.array(
        [np.float32(np.float16(np.float32(m) * c04 - 1.0)) for m in range(M1)],
        np.float64)
    V = np.vander(nodes, M1, increasing=True)  # V[m, kk] = w_m^kk
    Vinv = np.linalg.inv(V)

    id16 = np.eye(128, dtype=np.float16)
    id32 = np.eye(128, dtype=np.float32)

    in_maps = []
    assert BS == 2 and q.shape[1] == SEQ and q.shape[2] == DM
    for c in range(8):
        b = c // 4
        h0 = 2 * (c % 4)
        wq_s = ln_gamma[:, None] * w_qs[:, h0 * 64:(h0 + 2) * 64] / np.sqrt(DK)
        chs = [wq_s[:, hl * 64:(hl + 1) * 64].astype(np.float64)
               @ base_rpr.T.astype(np.float64) @ Vinv.T for hl in range(HPC)]
        wq_ext = np.concatenate([wq_s] + [c_.astype(np.float32) for c_ in chs],
                                axis=1)
        m = {
            "qx": q[b], "kx": k[b], "vx": v[b], "dist": dist[b],
            "wq": wq_ext.astype(np.float16),
            "wk": w_ks[:, h0 * 64:(h0 + 2) * 64].astype(np.float16),
            "wv": w_vs[:, h0 * 64:(h0 + 2) * 64].astype(np.float16),
            "wfc": w_fc[h0 * 64:(h0 + 2) * 64, :].astype(np.float16),
            "id16": id16, "id32": id32,
        }
        if use_mask:
            m["maskp"] = mask[b]
        if use_beta:
            m["qbias"] = (ln_beta @ wq_ext).astype(np.float32)[:, None]
        in_maps.append(m)

    nc = _get_program(use_mask, use_beta)
    res = run_bass_kernel_spmd(nc, in_maps, list(range(8)))

    attn = np.empty((BS, NH, SEQ, SEQ), np.float32)
    out = np.empty((BS, SEQ, DM), np.float32)
    for b in range(BS):
        acc = None
        for g in range(4):
            c = b * 4 + g
            attn[b, 2 * g:2 * g + 2] = res.results[c]["attn_o"]
            part = res.results[c]["out_o"]
            acc = part if acc is None else acc + part
        out[b] = acc.T + q[b]
    return out, attn


if __name__ == "__main__":
    rng = np.random.default_rng(0)
    inp = {
        'q': rng.standard_normal((2, SEQ, DM), np.float32),
        'k': rng.standard_normal((2, SEQ, DM), np.float32),
        'v': rng.standard_normal((2, SEQ, DM), np.float32),
        'mask': np.ones((2, SEQ, SEQ), np.int32),
        'dist': rng.integers(0, 10, (2, SEQ, SEQ)).astype(np.int32),
        'w_qs': (rng.standard_normal((DM, NH * DK), np.float32) * 0.02),
        'w_ks': (rng.standard_normal((DM, NH * DK), np.float32) * 0.02),
        'w_vs': (rng.standard_normal((DM, NH * DK), np.float32) * 0.02),
        'w_fc': (rng.standard_normal((NH * DK, DM), np.float32) * 0.02),
        'ln_gamma': np.ones(DM, np.float32),
        'ln_beta': np.zeros(DM, np.float32),
        'base_rpr': (rng.standard_normal((M1, DK), np.float32) * 0.02),
    }
    out, attn = kernel(**inp)
    print("out", out.shape, "attn", attn.shape)


# revision 8
# speedup vs baseline: 1.8261x; 1.8261x over previous
# Trainium2 Bass kernel for nn_MultiHeadAttention_77120432767767
# (relative-position sparse attention).
#
# Sharding: batch (2) x head-pairs (4) across 8 cores. Core c handles
# batch b=c//4, heads h0=2*(c%4), h0+1. base_rpr / projection weights are
# folded on the host into per-core weight slices; dist is sharded with batch.
#
# Key trick: the relative-position gather attn2[i,j] = qb[i, min(dist[i,j],5)]
# is evaluated as a degree-5 polynomial in w = (min(dist,5) - 2.5)/2.5
# (Chebyshev-style nodes in [-1,1] so fp16 rounding does not amplify), whose
# per-row coefficients come out of the q-projection matmul for free: the
# Vandermonde inverse is folded into the projection weights on the host.
import sys
import os

sys.path.insert(0, '/opt/trn_rl_repo')

import numpy as np
from contextlib import ExitStack

import concourse.bass as bass
import concourse.tile as tile
from concourse import mybir
from concourse.bass_utils import run_bass_kernel_spmd

F32 = mybir.dt.float32
F16 = mybir.dt.float16
I32 = mybir.dt.int32
AL = mybir.AluOpType
AF = mybir.ActivationFunctionType

SEQ, DM, NH, DK, M1 = 1024, 512, 8, 64, 6
HPC = 2           # heads per core
NT = SEQ // 128   # 8 seq tiles
NDT = DM // 128   # 4 model-dim tiles

# ---------------------------------------------------------------------------
# walrus in this container only accepts ONE sync-wait per ctrl instruction;
# TileContext's tail drain attaches one wait per outstanding semaphore.
# Spread them across single-wait NOPs on SP instead.
_orig_drain_and_barrier = tile.TileContext._drain_and_barrier


def _patched_drain_and_barrier(self, tick_clock, wait_clock):
    collector = self.nc.sync.nop()
    wait_clock.add_sem_waits(
        collector.ins, tile.ScopedClock({None: tick_clock.global_clock})
    )
    si = collector.ins.sync_info
    waits = list(si.on_wait or []) if si else []
    if si:
        si.on_wait = waits[:1]
    for i in range(1, len(waits)):
        n = self.nc.sync.nop()
        n.ins.sync_info = mybir.SyncInfo(on_wait=[waits[i]], on_update=[])
    self.nc.sync.drain()
    self.nc.all_engine_barrier()
    popped = self.nc._tile_sem_poison_stack.pop()
    assert popped is self._sem_poison
    self.nc.clear_and_free_semaphores(list(self.sems.allocated().values()))
    self.nc.all_engine_barrier()


tile.TileContext._drain_and_barrier = _patched_drain_and_barrier

_wsplit_ctr = [0]


def _split_multiwait_instructions(nc):
    """walrus here accepts one sync-wait per instruction; hoist extras onto
    same-engine NOPs inserted immediately before (same stream => same order)."""
    for bassbb in nc.bb_map.values():
        bb = bassbb.bb
        insts = list(bb.instructions)
        new, changed = [], False
        for inst in insts:
            si = inst.sync_info
            if si is not None and si.on_wait and len(si.on_wait) > 1:
                waits = list(si.on_wait)
                for w in waits[:-1]:
                    nop = mybir.InstNoOp(
                        name=f"I-wsplit{_wsplit_ctr[0]}", ins=[], outs=[])
                    _wsplit_ctr[0] += 1
                    nop.engine = inst.engine
                    nop.sync_info = mybir.SyncInfo(on_wait=[w], on_update=[])
                    nc.register_instruction(nop)
                    new.append(nop)
                si.on_wait = [waits[-1]]
                changed = True
            new.append(inst)
        if changed:
            bb.instructions = new


# ---------------------------------------------------------------------------
def build_program(use_mask: bool, use_beta: bool):
    nc = bass.Bass()
    qx = nc.declare_dram_parameter("qx", [SEQ, DM], F32, isOutput=False)
    kx = nc.declare_dram_parameter("kx", [SEQ, DM], F32, isOutput=False)
    vx = nc.declare_dram_parameter("vx", [SEQ, DM], F32, isOutput=False)
    dist = nc.declare_dram_parameter("dist", [SEQ, SEQ], I32, isOutput=False)
    wq = nc.declare_dram_parameter("wq", [DM, 128 + HPC * M1], F16, isOutput=False)
    wk = nc.declare_dram_parameter("wk", [DM, 128], F16, isOutput=False)
    wv = nc.declare_dram_parameter("wv", [DM, 128], F16, isOutput=False)
    wfc = nc.declare_dram_parameter("wfc", [128, DM], F16, isOutput=False)
    id16 = nc.declare_dram_parameter("id16", [128, 128], F16, isOutput=False)
    id32 = nc.declare_dram_parameter("id32", [128, 128], F32, isOutput=False)
    maskp = None
    if use_mask:
        maskp = nc.declare_dram_parameter("maskp", [SEQ, SEQ], I32, isOutput=False)
    qbias = None
    if use_beta:
        # beta @ wq_ext : per-output-channel bias for qs rows / coef rows
        qbias = nc.declare_dram_parameter(
            "qbias", [128 + HPC * M1, 1], F32, isOutput=False
        )

    attn_o = nc.declare_dram_parameter("attn_o", [HPC, SEQ, SEQ], F32, isOutput=True)
    out_o = nc.declare_dram_parameter("out_o", [DM, SEQ], F32, isOutput=True)

    with tile.TileContext(nc) as tc, ExitStack() as ctx:
        body(ctx, tc, qx, kx, vx, dist, wq, wk, wv, wfc, id16, id32,
             maskp, qbias, attn_o, out_o)
    _split_multiwait_instructions(nc)
    return nc


def body(ctx, tc, qx, kx, vx, dist, wq, wk, wv, wfc, id16, id32,
         maskp, qbias, attn_o, out_o):
    nc = tc.nc

    singles = ctx.enter_context(tc.tile_pool(name="singles", bufs=1))
    ld = ctx.enter_context(tc.tile_pool(name="ld", bufs=3))          # f32 input tiles
    cv = ctx.enter_context(tc.tile_pool(name="cv", bufs=3))          # f16 converted tiles
    stats = ctx.enter_context(tc.tile_pool(name="stats", bufs=4))
    big = ctx.enter_context(tc.tile_pool(name="big", bufs=1))        # resident tensors
    work = ctx.enter_context(tc.tile_pool(name="work", bufs=3))      # horner/x/w tiles
    pwork = ctx.enter_context(tc.tile_pool(name="pwork", bufs=3))    # pu / pn tiles
    # PSUM budget: psA "psbig" [128,1024]f32 = 2 banks x 2 bufs = 4 banks;
    # psB "pssmall" up to 4KB/partition = 2 banks x 2 bufs = 4 banks. Total 8.
    psA = ctx.enter_context(tc.tile_pool(name="psA", bufs=2, space="PSUM"))
    psB = ctx.enter_context(tc.tile_pool(name="psB", bufs=2, space="PSUM"))

    # identities
    ident16 = singles.tile([128, 128], F16, tag="ident16")
    nc.sync.dma_start(out=ident16, in_=id16[:, :])
    ident32 = singles.tile([128, 128], F32, tag="ident32")
    nc.sync.dma_start(out=ident32, in_=id32[:, :])

    # weights: [128, 4, E] layout: partition = dm % 128, free = (dm//128, e)
    wq_sb = singles.tile([128, NDT, 128 + HPC * M1], F16, tag="wq")
    nc.sync.dma_start(out=wq_sb, in_=wq.rearrange("(t p) e -> p t e", p=128))
    wk_sb = singles.tile([128, NDT, 128], F16, tag="wk")
    nc.sync.dma_start(out=wk_sb, in_=wk.rearrange("(t p) e -> p t e", p=128))
    wv_sb = singles.tile([128, NDT, 128], F16, tag="wv")
    nc.sync.dma_start(out=wv_sb, in_=wv.rearrange("(t p) e -> p t e", p=128))
    wfc_sb = singles.tile([128, DM], F16, tag="wfc")
    nc.sync.dma_start(out=wfc_sb, in_=wfc[:, :])
    qbias_sb = None
    if qbias is not None:
        qbias_sb = singles.tile([128 + HPC * M1, 1], F32, tag="qbias")
        nc.sync.dma_start(out=qbias_sb, in_=qbias[:, :])

    # resident transposed activations: [128, dm_tile, seq]
    qnT = big.tile([128, NDT, SEQ], F16, tag="qnT")
    kT = big.tile([128, NDT, SEQ], F16, tag="kT")
    vT = big.tile([128, NDT, SEQ], F16, tag="vT")

    # ---- load q, LN stats, qn16, transpose ----
    for t in range(NT):
        qt = ld.tile([128, DM], F32, tag="qld", bufs=2)
        nc.sync.dma_start(out=qt, in_=qx[t * 128:(t + 1) * 128, :])
        st = stats.tile([128, nc.vector.BN_STATS_DIM], F32, tag="bst")
        nc.vector.bn_stats(out=st, in_=qt)
        mv = stats.tile([128, nc.vector.BN_AGGR_DIM], F32, tag="bagg")
        nc.vector.bn_aggr(out=mv, in_=st)
        varp = stats.tile([128, 1], F32, tag="varp")
        nc.vector.tensor_scalar_add(varp, mv[:, 1:2], 1e-6)
        sd = stats.tile([128, 1], F32, tag="sd")
        nc.scalar.sqrt(sd, varp)
        rstd = stats.tile([128, 1], F32, tag="rstd")
        nc.vector.reciprocal(rstd, sd)
        qn = cv.tile([128, DM], F16, tag="qn", bufs=2)
        nc.vector.tensor_scalar(qn, qt, mv[:, 0:1], rstd, AL.subtract, AL.mult)
        # transpose 4 chunks -> qnT[:, :, t*128...]
        pt = psB.tile([128, DM], F16, tag="pssmall")
        for c in range(NDT):
            nc.tensor.transpose(pt[:, c * 128:(c + 1) * 128],
                                qn[:, c * 128:(c + 1) * 128], ident16)
        nc.scalar.copy(qnT[:, :, t * 128:(t + 1) * 128],
                       pt.rearrange("p (c s) -> p c s", c=NDT))

    # ---- load k, v, convert, transpose ----
    for t in range(NT):
        kt = ld.tile([128, DM], F32, tag="kld", bufs=2)
        nc.sync.dma_start(out=kt, in_=kx[t * 128:(t + 1) * 128, :])
        k16 = cv.tile([128, DM], F16, tag="k16", bufs=2)
        nc.scalar.copy(k16, kt)
        pt = psB.tile([128, DM], F16, tag="pssmall")
        for c in range(NDT):
            nc.tensor.transpose(pt[:, c * 128:(c + 1) * 128],
                                k16[:, c * 128:(c + 1) * 128], ident16)
        nc.scalar.copy(kT[:, :, t * 128:(t + 1) * 128],
                       pt.rearrange("p (c s) -> p c s", c=NDT))
    for t in range(NT):
        vt = ld.tile([128, DM], F32, tag="vld", bufs=2)
        nc.sync.dma_start(out=vt, in_=vx[t * 128:(t + 1) * 128, :])
        v16 = cv.tile([128, DM], F16, tag="v16", bufs=2)
        nc.scalar.copy(v16, vt)
        pt = psB.tile([128, DM], F16, tag="pssmall")
        for c in range(NDT):
            nc.tensor.transpose(pt[:, c * 128:(c + 1) * 128],
                                v16[:, c * 128:(c + 1) * 128], ident16)
        nc.scalar.copy(vT[:, :, t * 128:(t + 1) * 128],
                       pt.rearrange("p (c s) -> p c s", c=NDT))

    # ---- projections ----
    NE = 128 + HPC * M1
    # q projection (+ polynomial coefficient columns)
    ps_q = psA.tile([128, SEQ], F32, tag="psbig")
    ps_c = psA.tile([M1 * HPC, SEQ], F32, tag="psbig")
    for c in range(NDT):
        for half in range(2):
            sl = slice(half * 512, (half + 1) * 512)
            nc.tensor.matmul(ps_q[:, sl], wq_sb[:, c, 0:128], qnT[:, c, sl],
                             start=(c == 0), stop=(c == NDT - 1))
            nc.tensor.matmul(ps_c[:, sl], wq_sb[:, c, 128:NE], qnT[:, c, sl],
                             start=(c == 0), stop=(c == NDT - 1))
    qsT = big.tile([128, SEQ], F16, tag="qsT")
    if qbias_sb is not None:
        nc.vector.tensor_scalar_add(qsT, ps_q, qbias_sb[0:128, :])
    else:
        nc.vector.tensor_copy(qsT, ps_q)
    aTf = big.tile([M1 * HPC, SEQ], F32, tag="aTf")
    if qbias_sb is not None:
        nc.vector.tensor_scalar_add(aTf, ps_c, qbias_sb[128:NE, :])
    else:
        nc.vector.tensor_copy(aTf, ps_c)
    # coefficients natural layout [seq, 12]
    a_sb = big.tile([128, NT, M1 * HPC], F32, tag="a_sb")
    for t in range(NT):
        pc = psB.tile([128, M1 * HPC], F32, tag="pssmall")
        nc.tensor.transpose(pc, aTf[:, t * 128:(t + 1) * 128],
                            ident32[0:M1 * HPC, 0:M1 * HPC])
        nc.vector.tensor_copy(a_sb[:, t, :], pc)

    # k projection
    ps_k = psA.tile([128, SEQ], F32, tag="psbig")
    for c in range(NDT):
        for half in range(2):
            sl = slice(half * 512, (half + 1) * 512)
            nc.tensor.matmul(ps_k[:, sl], wk_sb[:, c, :], kT[:, c, sl],
                             start=(c == 0), stop=(c == NDT - 1))
    khT = big.tile([128, SEQ], F16, tag="khT")
    nc.vector.tensor_copy(khT, ps_k)

    # v projection -> vh natural [seq, 128] stored as [128, jt, 128]
    vh = big.tile([128, NT, 128], F16, tag="vh")
    for jt in range(NT):
        ps_v = psB.tile([128, 128], F32, tag="pssmall")
        for c in range(NDT):
            nc.tensor.matmul(ps_v, vT[:, c, jt * 128:(jt + 1) * 128],
                             wv_sb[:, c, :],
                             start=(c == 0), stop=(c == NDT - 1))
        nc.vector.tensor_copy(vh[:, jt, :], ps_v)

    # ---- main attention loop ----
    c04 = float(np.float16(0.4))
    pT = [big.tile([128, NT, SEQ], F16, tag=f"pT{h}", name=f"pT{h}") for h in range(HPC)]
    rstore = [big.tile([128, NT], F32, tag=f"rst{h}", name=f"rst{h}") for h in range(HPC)]

    for i in range(NT):
        isl = slice(i * 128, (i + 1) * 128)
        dt_ = ld.tile([128, SEQ], I32, tag="dld", bufs=2)
        nc.sync.dma_start(out=dt_, in_=dist[isl, :])
        x16 = work.tile([128, SEQ], F16, tag="x16", bufs=2)
        nc.vector.tensor_scalar_min(x16, dt_, 5)
        w16 = work.tile([128, SEQ], F16, tag="w16", bufs=2)
        nc.vector.tensor_scalar(w16, x16, c04, 1.0, AL.mult, AL.subtract)
        w2 = work.tile([128, SEQ], F16, tag="w2", bufs=2)
        nc.vector.tensor_mul(w2, w16, w16)
        mk = None
        if maskp is not None:
            mt = ld.tile([128, SEQ], I32, tag="mld", bufs=2)
            nc.sync.dma_start(out=mt, in_=maskp[isl, :])
            mk = work.tile([128, SEQ], F16, tag="mk16", bufs=2)
            # (mask * 1e9) - 1e9 -> 0 where mask==1, -1e9 where mask==0
            nc.vector.tensor_scalar(mk, mt, 1e9, 1e9, AL.mult, AL.subtract)

        for h in range(HPC):
            hsl = slice(h * 64, (h + 1) * 64)
            ac = lambda j: a_sb[:, i, h * M1 + j:h * M1 + j + 1]
            ps_l = psA.tile([128, SEQ], F32, tag="psbig")
            for half in range(2):
                sl = slice(half * 512, (half + 1) * 512)
                nc.tensor.matmul(ps_l[:, sl], qsT[hsl, isl], khT[hsl, sl],
                                 start=True, stop=False, skip_group_check=True)
            # P(w) = t1 + w2*(t2 + w2*t3), t_k = affine in w (ACT Identity)
            t1 = work.tile([128, SEQ], F16, tag="poly", bufs=8)
            nc.scalar.activation(t1, w16, AF.Identity, bias=ac(0), scale=ac(1))
            t2 = work.tile([128, SEQ], F16, tag="poly", bufs=8)
            nc.scalar.activation(t2, w16, AF.Identity, bias=ac(2), scale=ac(3))
            t3 = work.tile([128, SEQ], F16, tag="poly", bufs=8)
            nc.scalar.activation(t3, w16, AF.Identity, bias=ac(4), scale=ac(5))
            u1 = work.tile([128, SEQ], F16, tag="poly", bufs=8)
            nc.vector.tensor_mul(u1, w2, t3)
            u2 = work.tile([128, SEQ], F16, tag="poly", bufs=8)
            nc.vector.tensor_add(u2, t2, u1)
            u3 = work.tile([128, SEQ], F16, tag="poly", bufs=8)
            nc.vector.tensor_mul(u3, w2, u2)
            for half in range(2):
                sl = slice(half * 512, (half + 1) * 512)
                nc.tensor.matmul(ps_l[:, sl], ident16, t1[:, sl],
                                 start=False, stop=False,
                                 skip_group_check=True)
                nc.tensor.matmul(ps_l[:, sl], ident16, u3[:, sl],
                                 start=False, stop=(mk is None),
                                 skip_group_check=True)
                if mk is not None:
                    nc.tensor.matmul(ps_l[:, sl], ident16, mk[:, sl],
                                     start=False, stop=True,
                                     skip_group_check=True)
            pu = pwork.tile([128, SEQ], F16, tag="pu", bufs=3)
            sums = stats.tile([128, 1], F32, tag="sums")
            nc.scalar.activation(pu, ps_l, AF.Exp, bias=0.0, scale=1.0,
                                 accum_out=sums)
            nc.vector.reciprocal(rstore[h][:, i:i + 1], sums)
            pn = pwork.tile([128, SEQ], F32, tag="pn", bufs=2)
            nc.scalar.activation(pn, pu, AF.Copy, bias=0.0,
                                 scale=rstore[h][:, i:i + 1])
            nc.gpsimd.dma_start(out=attn_o[h, isl, :], in_=pn)
            # transpose pu -> pT[h][:, jt, i*128...]
            for g in range(2):
                pt = psB.tile([128, 512], F16, tag="pssmall")
                for c in range(4):
                    j0 = (g * 4 + c) * 128
                    nc.tensor.transpose(pt[:, c * 128:(c + 1) * 128],
                                        pu[:, j0:j0 + 128], ident16)
                if g == 0:
                    nc.vector.tensor_copy(
                        pT[h][:, g * 4:(g + 1) * 4, isl],
                        pt.rearrange("p (c s) -> p c s", c=4))
                else:
                    nc.scalar.copy(
                        pT[h][:, g * 4:(g + 1) * 4, isl],
                        pt.rearrange("p (c s) -> p c s", c=4))

    # ---- PV + normalize + FC ----
    ones64 = singles.tile([1, 64], F32, tag="ones64")
    nc.vector.memset(ones64, 1.0)
    aoT = big.tile([128, SEQ], F16, tag="aoT")
    for h in range(HPC):
        hsl = slice(h * 64, (h + 1) * 64)
        # rT: transpose rstore[h] columns -> [1, SEQ]
        ps_rt = psB.tile([1, SEQ], F32, tag="pssmall")
        for t in range(NT):
            nc.tensor.transpose(ps_rt[0:1, t * 128:(t + 1) * 128],
                                rstore[h][:, t:t + 1], ident32)
        rT = stats.tile([1, SEQ], F32, tag="rT")
        nc.vector.tensor_copy(rT, ps_rt)
        ps_R = psA.tile([64, SEQ], F32, tag="psbig")
        for half in range(2):
            sl = slice(half * 512, (half + 1) * 512)
            nc.tensor.matmul(ps_R[:, sl], ones64, rT[:, sl],
                             start=True, stop=True)
        Rh = pwork.tile([64, SEQ], F32, tag="Rh", bufs=2)
        nc.vector.tensor_copy(Rh, ps_R)
        ps_ao = psA.tile([64, SEQ], F32, tag="psbig")
        for jt in range(NT):
            for half in range(2):
                sl = slice(half * 512, (half + 1) * 512)
                nc.tensor.matmul(ps_ao[:, sl], vh[:, jt, hsl],
                                 pT[h][:, jt, sl],
                                 start=(jt == 0), stop=(jt == NT - 1))
        nc.vector.tensor_mul(aoT[hsl, :], ps_ao, Rh)

    for sl4 in range(NDT):
        ps_f = psA.tile([128, SEQ], F32, tag="psbig")
        for half in range(2):
            sl = slice(half * 512, (half + 1) * 512)
            nc.tensor.matmul(ps_f[:, sl], wfc_sb[:, sl4 * 128:(sl4 + 1) * 128],
                             aoT[:, sl], start=True, stop=True)
        of = pwork.tile([128, SEQ], F32, tag="of", bufs=2)
        nc.scalar.copy(of, ps_f)
        nc.gpsimd.dma_start(out=out_o[sl4 * 128:(sl4 + 1) * 128, :], in_=of)


# ---------------------------------------------------------------------------
_PROGRAM_CACHE = {}


def _get_program(use_mask, use_beta):
    key = (use_mask, use_beta)
    if key not in _PROGRAM_CACHE:
        _PROGRAM_CACHE[key] = build_program(use_mask, use_beta)
    return _PROGRAM_CACHE[key]


def kernel(q, k, v, mask, dist, w_qs, w_ks, w_vs, w_fc, ln_gamma, ln_beta,
           base_rpr):
    q = np.asarray(q, np.float32)
    k = np.asarray(k, np.float32)
    v = np.asarray(v, np.float32)
    mask = np.asarray(mask, np.int32)
    dist = np.asarray(dist, np.int32)
    w_qs = np.asarray(w_qs, np.float32)
    w_ks = np.asarray(w_ks, np.float32)
    w_vs = np.asarray(w_vs, np.float32)
    w_fc = np.asarray(w_fc, np.float32)
    ln_gamma = np.asarray(ln_gamma, np.float32)
    ln_beta = np.asarray(ln_beta, np.float32)
    base_rpr = np.asarray(base_rpr, np.float32)

    BS = q.shape[0]
    use_mask = not bool(np.all(mask == 1))
    use_beta = bool(np.any(ln_beta != 0.0))

    # device-exact Chebyshev-ish nodes: w_m = fp16(fp16(m) * fp16(0.4) - 1)
    c04 = np.float32(np.float16(0.4))
    nodes = np.array(
        [np.float32(np.float16(np.float32(m) * c04 - 1.0)) for m in range(M1)],
        np.float64)
    V = np.vander(nodes, M1, increasing=True)  # V[m, kk] = w_m^kk
    Vinv = np.linalg.inv(V)

    id16 = np.eye(128, dtype=np.float16)
    id32 = np.eye(128, dtype=np.float32)

    in_maps = []
    assert BS == 2 and q.shape[1] == SEQ and q.shape[2] == DM
    for c in range(8):
        b = c // 4
        h0 = 2 * (c % 4)
        wq_s = ln_gamma[:, None] * w_qs[:, h0 * 64:(h0 + 2) * 64] / np.sqrt(DK)
        chs = [wq_s[:, hl * 64:(hl + 1) * 64].astype(np.float64)
               @ base_rpr.T.astype(np.float64) @ Vinv.T for hl in range(HPC)]
        wq_ext = np.concatenate([wq_s] + [c_.astype(np.float32) for c_ in chs],
                                axis=1)
        m = {
            "qx": q[b], "kx": k[b], "vx": v[b], "dist": dist[b],
            "wq": wq_ext.astype(np.float16),
            "wk": w_ks[:, h0 * 64:(h0 + 2) * 64].astype(np.float16),
            "wv": w_vs[:, h0 * 64:(h0 + 2) * 64].astype(np.float16),
            "wfc": w_fc[h0 * 64:(h0 + 2) * 64, :].astype(np.float16),
            "id16": id16, "id32": id32,
        }
        if use_mask:
            m["maskp"] = mask[b]
        if use_beta:
            m["qbias"] = (ln_beta @ wq_ext).astype(np.float32)[:, None]
        in_maps.append(m)

    nc = _get_program(use_mask, use_beta)
    res = run_bass_kernel_spmd(nc, in_maps, list(range(8)))

    attn = np.empty((BS, NH, SEQ, SEQ), np.float32)
    out = np.empty((BS, SEQ, DM), np.float32)
    for b in range(BS):
        acc = None
        for g in range(4):
            c = b * 4 + g
            attn[b, 2 * g:2 * g + 2] = res.results[c]["attn_o"]
            part = res.results[c]["out_o"]
            acc = part if acc is None else acc + part
        out[b] = acc.T + q[b]
    return out, attn


if __name__ == "__main__":
    rng = np.random.default_rng(0)
    inp = {
        'q': rng.standard_normal((2, SEQ, DM), np.float32),
        'k': rng.standard_normal((2, SEQ, DM), np.float32),
        'v': rng.standard_normal((2, SEQ, DM), np.float32),
        'mask': np.ones((2, SEQ, SEQ), np.int32),
        'dist': rng.integers(0, 10, (2, SEQ, SEQ)).astype(np.int32),
        'w_qs': (rng.standard_normal((DM, NH * DK), np.float32) * 0.02),
        'w_ks': (rng.standard_normal((DM, NH * DK), np.float32) * 0.02),
        'w_vs': (rng.standard_normal((DM, NH * DK), np.float32) * 0.02),
        'w_fc': (rng.standard_normal((NH * DK, DM), np.float32) * 0.02),
        'ln_gamma': np.ones(DM, np.float32),
        'ln_beta': np.zeros(DM, np.float32),
        'base_rpr': (rng.standard_normal((M1, DK), np.float32) * 0.02),
    }
    out, attn = kernel(**inp)
    print("out", out.shape, "attn", attn.shape)
